# revision 6
# baseline (speedup 1.0000x reference)
"""Block-sparse linear (BlockSparseLinear) Trainium2 kernel.

Math: out[B, OUT_F] = block_sparse_matmul(x) + bias, with 16x16 blocks in CSR
form (block_row_indices row pointers, block_col_indices per-nonzero column).

Strategy (8 NeuronCores, data-parallel over tokens):
- Each core handles 256 tokens: full x^T slice [4096, 256] fp32 DMA'd in,
  cast once to fp16, then replicated into 4 partition-rolled SBUF images so
  any 32-feature window is available at every 32-aligned partition base.
- Weights: host merges duplicate (row, col) blocks, groups each row's blocks
  into 32-feature windows (column pairs), packs them as fp16 [32,16] lhsT
  slots into a per-flight weight image (identical on all cores).
- Compute: PE 32x32 tile packing. 16 flights x 16 rows; each output block-row
  is owned by exactly one (row_grp i, col_grp j) PE tile accumulating into its
  own PSUM slice (bank i of the flight's bank set, partitions [32j, 32j+16)).
  One MM per window: [K=32, M=16, N=256] fp16 -> fp32 PSUM.
- Drain: ScalarE copy+bias PSUM -> SBUF, then DMA each row slice to the
  per-core out^T [4096, 256] fp32. Host concatenates and transposes.
"""

import numpy as np

BLOCK = 16
IN_F = 4096
OUT_F = 4096
TOKENS = 2048
NCORES = 8
NTOK = TOKENS // NCORES          # 256 tokens per core
NBR = OUT_F // BLOCK             # 256 block rows
NBC = IN_F // BLOCK              # 256 block cols
NWIN = NBC // 2                  # 128 windows (adjacent col pairs)
NXT = IN_F // 128                # 32 x-tiles of 128 features
ROWS_PER_FLIGHT = 16
NFLIGHTS = NBR // ROWS_PER_FLIGHT  # 16


def _preprocess(sparse_blocks, block_row_indices, block_col_indices, bias):
    """Merge duplicate blocks, build window schedule + packed weight image."""
    nnz = sparse_blocks.shape[0]
    row_ids = (
        np.searchsorted(
            block_row_indices, np.arange(nnz, dtype=block_row_indices.dtype),
            side="right",
        ) - 1
    )
    cols = block_col_indices.astype(np.int64)

    # merge duplicates: windows[r][m] = [W_even or None, W_odd or None]
    windows = [dict() for _ in range(NBR)]
    for idx in range(nnz):
        r = int(row_ids[idx])
        c = int(cols[idx])
        m, h = c // 2, c % 2
        slot = windows[r].setdefault(m, [None, None])
        if slot[h] is None:
            slot[h] = sparse_blocks[idx].astype(np.float32)
        else:
            slot[h] = slot[h] + sparse_blocks[idx]

    counts = np.array([len(windows[r]) for r in range(NBR)])
    order = np.argsort(-counts, kind="stable")  # heavy rows first

    # flights of 16 rows; within a flight, position idx -> tile (i=idx%4, j=idx//4)
    sched = []        # per flight: list of (i, j, r, [(s, g, t), ...])
    s_max = 0
    flight_rows = []
    for f in range(NFLIGHTS):
        rows = order[f * 16:(f + 1) * 16]
        offs = [0, 0, 0, 0]
        tiles = []
        for idx, r in enumerate(rows):
            i, j = idx % 4, idx // 4
            slots = []
            for m in sorted(windows[r].keys()):
                s = offs[i]
                offs[i] += 1
                g = (i - (m % 4)) % 4     # image whose roll puts window m at base 32i
                t = m // 4                # x-tile (256-token column block)
                slots.append((s, g, t, m))
            tiles.append((i, j, int(r), slots))
        sched.append(tiles)
        flight_rows.append(rows)
        s_max = max(s_max, max(offs))

    CW = 16 * s_max
    w_host = np.zeros((NFLIGHTS, 128, CW), dtype=np.float16)
    for f, tiles in enumerate(sched):
        for (i, j, r, slots) in tiles:
            for (s, g, t, m) in slots:
                pair = windows[r][m]
                blk = np.zeros((32, 16), dtype=np.float32)
                if pair[0] is not None:
                    blk[0:16] = pair[0].T     # lhsT = W_block^T  (W is [out, in])
                if pair[1] is not None:
                    blk[16:32] = pair[1].T
                w_host[f, 32 * i:32 * i + 32, 16 * s:16 * s + 16] = (
                    blk.astype(np.float16))

    # bias image: col (4f + i), partition 32j+u  -> bias[16*r + u] for the row
    # owned by tile (i, j) of flight f
    bias_img = np.zeros((128, 4 * NFLIGHTS), dtype=np.float32)
    for f, tiles in enumerate(sched):
        for (i, j, r, slots) in tiles:
            bias_img[32 * j:32 * j + 16, 4 * f + i] = bias[16 * r:16 * r + 16]

    return sched, w_host, bias_img


def _build_program(sched, CW):
    import concourse.mybir as mybir
    from concourse import tile, bacc

    DT = mybir.dt.float32
    DTH = mybir.dt.float16

    nc = bacc.Bacc(None, target_bir_lowering=False, debug=False)
    x_dram = nc.dram_tensor("x", [IN_F, NTOK], DT, kind="ExternalInput")
    w_dram = nc.dram_tensor("w", [NFLIGHTS, 128, CW], DTH, kind="ExternalInput")
    b_dram = nc.dram_tensor("bias", [128, 4 * NFLIGHTS], DT, kind="ExternalInput")
    out_dram = nc.dram_tensor("out", [OUT_F, NTOK], DT, kind="ExternalOutput")

    with tile.TileContext(nc) as tc:
        with (
            tc.tile_pool(name="xpool", bufs=1) as xpool,
            tc.tile_pool(name="wpool", bufs=3) as wpool,
            tc.tile_pool(name="opool", bufs=8) as opool,
            tc.tile_pool(name="psum", bufs=1, space="PSUM") as psum_pool,
        ):
            stage = xpool.tile([128, NXT * NTOK], DT, tag="stage")
            imgs = [xpool.tile([128, NXT * NTOK], DTH, tag=f"img{g}", name=f"img{g}")
                    for g in range(4)]
            bias_sb = xpool.tile([128, 4 * NFLIGHTS], DT, tag="bias")

            nc.sync.dma_start(bias_sb[:], b_dram[:])
            # x^T [4096, 256] -> stage [128, 32*256]: feat 128t+p -> (p, 256t+tok)
            x_re = x_dram[:].rearrange("(t p) n -> p t n", p=128)
            stage_re = stage[:].rearrange("p (t n) -> p t n", t=NXT)
            for cc in range(4):
                nc.sync.dma_start(
                    stage_re[:, cc * 8:(cc + 1) * 8, :],
                    x_re[:, cc * 8:(cc + 1) * 8, :],
                )
            # fp32 -> fp16 natural image
            nc.vector.tensor_copy(imgs[0][:], stage[:])
            # partition-rolled images: img_g[p] = img_0[(p - 32g) % 128]
            for g in range(1, 4):
                nc.sync.dma_start(imgs[g][32 * g:128, :], imgs[0][0:128 - 32 * g, :])
                nc.sync.dma_start(imgs[g][0:32 * g, :], imgs[0][128 - 32 * g:128, :])

            psum_tiles = [psum_pool.tile([128, NTOK], DT, tag=f"ps{b}", name=f"ps{b}")
                          for b in range(8)]
            for pt in psum_tiles:
                nc.vector.memset(pt[:], 0.0)

            for f, tiles in enumerate(sched):
                bankset = 4 * (f % 2)
                wt = wpool.tile([128, CW], DTH, tag="w")
                nc.sync.dma_start(wt[:], w_dram[f])

                nrounds = max(len(slots) for (_, _, _, slots) in tiles)
                for k in range(nrounds):
                    for (i, j, r, slots) in tiles:
                        if k >= len(slots):
                            continue
                        s, g, t, m = slots[k]
                        nc.tensor.matmul(
                            psum_tiles[bankset + i][32 * j:32 * j + 16, :],
                            wt[32 * i:32 * i + 32, 16 * s:16 * s + 16],
                            imgs[g][32 * i:32 * i + 32, NTOK * t:NTOK * (t + 1)],
                            start=(k == 0), stop=(k == len(slots) - 1),
                            skip_group_check=True,
                            tile_position=(32 * i, 32 * j),
                        )

                # drain: one ACT op per bank (adds bias), then per-row DMA out
                for i in range(4):
                    ot = opool.tile([128, NTOK], DT, tag="o")
                    nc.scalar.activation(
                        ot[:], psum_tiles[bankset + i][:],
                        mybir.ActivationFunctionType.Identity,
                        bias=bias_sb[:, 4 * f + i:4 * f + i + 1],
                    )
                    for (ii, j, r, slots) in tiles:
                        if ii != i:
                            continue
                        nc.sync.dma_start(
                            out_dram[16 * r:16 * r + 16, :],
                            ot[32 * j:32 * j + 16, :],
                        )

    nc.compile()
    return nc


def _axon_usable():
    """True if this process can still reach the axon/neuron jax backend."""
    import os
    jp = os.environ.get("JAX_PLATFORMS")
    if jp is not None and "axon" not in jp and jp not in ("", None):
        import sys
        if "jax" in sys.modules:
            try:
                import jax
                return any(
                    d.platform not in ("cpu",) for d in jax.devices())
            except Exception:
                return False
        # jax not imported yet: clear the pin so the plugin is discoverable
        del os.environ["JAX_PLATFORMS"]
    try:
        import jax
        return any(d.platform not in ("cpu",) for d in jax.devices())
    except Exception:
        return False


def kernel(x, sparse_blocks, block_row_indices, block_col_indices, bias):
    x = np.asarray(x, dtype=np.float32)
    sparse_blocks = np.asarray(sparse_blocks, dtype=np.float32)
    block_row_indices = np.asarray(block_row_indices, dtype=np.int32)
    block_col_indices = np.asarray(block_col_indices, dtype=np.int32)
    bias = np.asarray(bias, dtype=np.float32)

    if not _axon_usable():
        # jax in this process is pinned to cpu; run on the devices from a
        # clean subprocess instead
        import os
        import subprocess
        import sys
        import tempfile
        tmpdir = tempfile.mkdtemp(prefix="bslin_")
        in_npz = os.path.join(tmpdir, "in.npz")
        out_npz = os.path.join(tmpdir, "out.npz")
        np.savez(in_npz, x=x, sparse_blocks=sparse_blocks,
                 block_row_indices=block_row_indices,
                 block_col_indices=block_col_indices, bias=bias)
        code = (
            "import numpy as np, importlib.util;"
            "spec = importlib.util.spec_from_file_location('bslin_kernel', %r);"
            "m = importlib.util.module_from_spec(spec);"
            "spec.loader.exec_module(m);"
            "d = dict(np.load(%r));"
            "np.savez(%r, out=m.kernel(**d))" % (__file__, in_npz, out_npz)
        )
        env = {k: v for k, v in os.environ.items() if k != "JAX_PLATFORMS"}
        subprocess.run([sys.executable, "-c", code], check=True, env=env)
        return np.load(out_npz)["out"]

    sched, w_host, bias_img = _preprocess(
        sparse_blocks, block_row_indices, block_col_indices, bias)
    CW = w_host.shape[2]
    nc = _build_program(sched, CW)

    in_maps = []
    for c in range(NCORES):
        xt = np.ascontiguousarray(x[NTOK * c:NTOK * (c + 1), :].T)
        in_maps.append({"x": xt, "w": w_host, "bias": bias_img})

    from concourse.bass_utils import run_bass_kernel_spmd
    res = run_bass_kernel_spmd(nc, in_maps, core_ids=list(range(NCORES)))

    out = np.empty((TOKENS, OUT_F), dtype=np.float32)
    for c in range(NCORES):
        out[NTOK * c:NTOK * (c + 1), :] = res.results[c]["out"].T
    return out


if __name__ == "__main__":
    rng = np.random.default_rng(0)
    x = rng.standard_normal((TOKENS, IN_F)).astype(np.float32)
    sb = (rng.standard_normal((8192, 16, 16)) * 0.02).astype(np.float32)
    bri = (np.arange(257) * 32).astype(np.int32)
    bci = np.sort(
        rng.integers(0, 256, size=(256, 32)), axis=1).reshape(-1).astype(np.int32)
    bias = (rng.standard_normal(4096) * 0.01).astype(np.float32)
    out = kernel(x, sb, bri, bci, bias)
    print("kernel ran, out shape", out.shape)


# revision 7
# speedup vs baseline: 1.9152x; 1.9152x over previous
"""Block-sparse linear (BlockSparseLinear) Trainium2 kernel.

Math: out[B, OUT_F] = block_sparse_matmul(x) + bias, with 16x16 blocks in CSR
form (block_row_indices row pointers, block_col_indices per-nonzero column).

Strategy (8 NeuronCores, data-parallel over tokens):
- Each core handles 256 tokens: full x^T slice [4096, 256] fp32 DMA'd in,
  cast once to fp16, then replicated into 4 partition-rolled SBUF images so
  any 32-feature window is available at every 32-aligned partition base.
- Weights: host merges duplicate (row, col) blocks, groups each row's blocks
  into 32-feature windows (column pairs), packs them as fp16 [32,16] lhsT
  slots into a per-flight weight image (identical on all cores).
- Compute: PE 32x32 tile packing. 16 flights x 16 rows; each output block-row
  is owned by exactly one (row_grp i, col_grp j) PE tile accumulating into its
  own PSUM slice (bank i of the flight's bank set, partitions [32j, 32j+16)).
  One MM per window: [K=32, M=16, N=256] fp16 -> fp32 PSUM.
- Drain: ScalarE copy+bias PSUM -> SBUF, then DMA each row slice to the
  per-core out^T [4096, 256] fp32. Host concatenates and transposes.
"""

import numpy as np

BLOCK = 16
IN_F = 4096
OUT_F = 4096
TOKENS = 2048
NCORES = 8
NTOK = TOKENS // NCORES          # 256 tokens per core
NBR = OUT_F // BLOCK             # 256 block rows
NBC = IN_F // BLOCK              # 256 block cols
NWIN = NBC // 2                  # 128 windows (adjacent col pairs)
NXT = IN_F // 128                # 32 x-tiles of 128 features
ROWS_PER_FLIGHT = 16
NFLIGHTS = NBR // ROWS_PER_FLIGHT  # 16


def _preprocess(sparse_blocks, block_row_indices, block_col_indices, bias):
    """Merge duplicate blocks, build window schedule + packed weight image."""
    nnz = sparse_blocks.shape[0]
    row_ids = (
        np.searchsorted(
            block_row_indices, np.arange(nnz, dtype=block_row_indices.dtype),
            side="right",
        ) - 1
    )
    cols = block_col_indices.astype(np.int64)

    # merge duplicates: windows[r][m] = [W_even or None, W_odd or None]
    windows = [dict() for _ in range(NBR)]
    for idx in range(nnz):
        r = int(row_ids[idx])
        c = int(cols[idx])
        m, h = c // 2, c % 2
        slot = windows[r].setdefault(m, [None, None])
        if slot[h] is None:
            slot[h] = sparse_blocks[idx].astype(np.float32)
        else:
            slot[h] = slot[h] + sparse_blocks[idx]

    counts = np.array([len(windows[r]) for r in range(NBR)])
    order = np.argsort(-counts, kind="stable")  # heavy rows first

    # flights of 16 rows; within a flight, position idx -> tile (i=idx%4, j=idx//4)
    sched = []        # per flight: list of (i, j, r, [(s, g, t), ...])
    s_max = 0
    flight_rows = []
    for f in range(NFLIGHTS):
        rows = order[f * 16:(f + 1) * 16]
        offs = [0, 0, 0, 0]
        tiles = []
        for idx, r in enumerate(rows):
            i, j = idx % 4, idx // 4
            slots = []
            for m in sorted(windows[r].keys()):
                s = offs[i]
                offs[i] += 1
                g = (i - (m % 4)) % 4     # image whose roll puts window m at base 32i
                t = m // 4                # x-tile (256-token column block)
                slots.append((s, g, t, m))
            tiles.append((i, j, int(r), slots))
        sched.append(tiles)
        flight_rows.append(rows)
        s_max = max(s_max, max(offs))

    CW = 16 * s_max
    w_host = np.zeros((NFLIGHTS, 128, CW), dtype=np.float16)
    for f, tiles in enumerate(sched):
        for (i, j, r, slots) in tiles:
            for (s, g, t, m) in slots:
                pair = windows[r][m]
                blk = np.zeros((32, 16), dtype=np.float32)
                if pair[0] is not None:
                    blk[0:16] = pair[0].T     # lhsT = W_block^T  (W is [out, in])
                if pair[1] is not None:
                    blk[16:32] = pair[1].T
                w_host[f, 32 * i:32 * i + 32, 16 * s:16 * s + 16] = (
                    blk.astype(np.float16))

    # bias image: col (4f + i), partition 32j+u  -> bias[16*r + u] for the row
    # owned by tile (i, j) of flight f
    bias_img = np.zeros((128, 4 * NFLIGHTS), dtype=np.float32)
    for f, tiles in enumerate(sched):
        for (i, j, r, slots) in tiles:
            bias_img[32 * j:32 * j + 16, 4 * f + i] = bias[16 * r:16 * r + 16]

    return sched, w_host, bias_img


def _build_program(sched, CW):
    import concourse.mybir as mybir
    from concourse import tile, bacc

    DT = mybir.dt.float32
    DTH = mybir.dt.float16

    nc = bacc.Bacc(None, target_bir_lowering=False, debug=False)
    x_dram = nc.dram_tensor("x", [IN_F, NTOK], DT, kind="ExternalInput")
    w_dram = nc.dram_tensor("w", [NFLIGHTS, 128, CW], DTH, kind="ExternalInput")
    b_dram = nc.dram_tensor("bias", [128, 4 * NFLIGHTS], DT, kind="ExternalInput")
    out_dram = nc.dram_tensor("out", [OUT_F, NTOK], DT, kind="ExternalOutput")

    with tile.TileContext(nc) as tc:
        with (
            tc.tile_pool(name="xpool", bufs=1) as xpool,
            tc.tile_pool(name="wpool", bufs=3) as wpool,
            tc.tile_pool(name="opool", bufs=8) as opool,
            tc.tile_pool(name="psum", bufs=1, space="PSUM") as psum_pool,
        ):
            stage = xpool.tile([128, NXT * NTOK], DT, tag="stage")
            imgs = [xpool.tile([128, NXT * NTOK], DTH, tag=f"img{g}", name=f"img{g}")
                    for g in range(4)]
            bias_sb = xpool.tile([128, 4 * NFLIGHTS], DT, tag="bias")

            nc.sync.dma_start(bias_sb[:], b_dram[:])
            # x^T [4096, 256] -> stage [128, 32*256]: feat 128t+p -> (p, 256t+tok)
            x_re = x_dram[:].rearrange("(t p) n -> p t n", p=128)
            stage_re = stage[:].rearrange("p (t n) -> p t n", t=NXT)
            for cc in range(4):
                nc.sync.dma_start(
                    stage_re[:, cc * 8:(cc + 1) * 8, :],
                    x_re[:, cc * 8:(cc + 1) * 8, :],
                )
            # fp32 -> fp16 natural image
            nc.vector.tensor_copy(imgs[0][:], stage[:])
            # partition-rolled images: img_g[p] = img_0[(p - 32g) % 128]
            for g in range(1, 4):
                nc.sync.dma_start(imgs[g][32 * g:128, :], imgs[0][0:128 - 32 * g, :])
                nc.sync.dma_start(imgs[g][0:32 * g, :], imgs[0][128 - 32 * g:128, :])

            psum_tiles = [psum_pool.tile([128, NTOK], DT, tag=f"ps{b}", name=f"ps{b}")
                          for b in range(8)]
            for pt in psum_tiles:
                nc.vector.memset(pt[:], 0.0)

            for f, tiles in enumerate(sched):
                bankset = 4 * (f % 2)
                wt = wpool.tile([128, CW], DTH, tag="w")
                nc.sync.dma_start(wt[:], w_dram[f])

                nrounds = max(len(slots) for (_, _, _, slots) in tiles)
                for k in range(nrounds):
                    for (i, j, r, slots) in tiles:
                        if k >= len(slots):
                            continue
                        s, g, t, m = slots[k]
                        nc.tensor.matmul(
                            psum_tiles[bankset + i][32 * j:32 * j + 16, :],
                            wt[32 * i:32 * i + 32, 16 * s:16 * s + 16],
                            imgs[g][32 * i:32 * i + 32, NTOK * t:NTOK * (t + 1)],
                            start=(k == 0), stop=(k == len(slots) - 1),
                            skip_group_check=True,
                            tile_position=(32 * i, 32 * j),
                        )

                # drain: one ACT op per bank (adds bias), then per-row DMA out
                for i in range(4):
                    ot = opool.tile([128, NTOK], DT, tag="o")
                    nc.scalar.activation(
                        ot[:], psum_tiles[bankset + i][:],
                        mybir.ActivationFunctionType.Identity,
                        bias=bias_sb[:, 4 * f + i:4 * f + i + 1],
                    )
                    for (ii, j, r, slots) in tiles:
                        if ii != i:
                            continue
                        nc.sync.dma_start(
                            out_dram[16 * r:16 * r + 16, :],
                            ot[32 * j:32 * j + 16, :],
                        )

    nc.compile()
    return nc


def _axon_usable():
    """True if this process can still reach the axon/neuron jax backend."""
    import os
    jp = os.environ.get("JAX_PLATFORMS")
    if jp is not None and "axon" not in jp and jp not in ("", None):
        import sys
        if "jax" in sys.modules:
            try:
                import jax
                return any(
                    d.platform not in ("cpu",) for d in jax.devices())
            except Exception:
                return False
        # jax not imported yet: clear the pin so the plugin is discoverable
        del os.environ["JAX_PLATFORMS"]
    try:
        import jax
        return any(d.platform not in ("cpu",) for d in jax.devices())
    except Exception:
        return False


def kernel(x, sparse_blocks, block_row_indices, block_col_indices, bias):
    x = np.asarray(x, dtype=np.float32)
    sparse_blocks = np.asarray(sparse_blocks, dtype=np.float32)
    block_row_indices = np.asarray(block_row_indices, dtype=np.int32)
    block_col_indices = np.asarray(block_col_indices, dtype=np.int32)
    bias = np.asarray(bias, dtype=np.float32)

    if not _axon_usable():
        # jax in this process is pinned to cpu; run on the devices from a
        # clean subprocess instead
        import os
        import subprocess
        import sys
        import tempfile
        tmpdir = tempfile.mkdtemp(prefix="bslin_")
        in_npz = os.path.join(tmpdir, "in.npz")
        out_npz = os.path.join(tmpdir, "out.npz")
        np.savez(in_npz, x=x, sparse_blocks=sparse_blocks,
                 block_row_indices=block_row_indices,
                 block_col_indices=block_col_indices, bias=bias)
        code = (
            "import numpy as np, importlib.util;"
            "spec = importlib.util.spec_from_file_location('bslin_kernel', %r);"
            "m = importlib.util.module_from_spec(spec);"
            "spec.loader.exec_module(m);"
            "d = dict(np.load(%r));"
            "np.savez(%r, out=m.kernel(**d))" % (__file__, in_npz, out_npz)
        )
        env = {k: v for k, v in os.environ.items() if k != "JAX_PLATFORMS"}
        subprocess.run([sys.executable, "-c", code], check=True, env=env)
        return np.load(out_npz)["out"]

    sched, w_host, bias_img = _preprocess(
        sparse_blocks, block_row_indices, block_col_indices, bias)
    CW = w_host.shape[2]
    nc = _build_program(sched, CW)

    in_maps = []
    for c in range(NCORES):
        xt = np.ascontiguousarray(x[NTOK * c:NTOK * (c + 1), :].T)
        in_maps.append({"x": xt, "w": w_host, "bias": bias_img})

    from concourse.bass_utils import run_bass_kernel_spmd
    res = run_bass_kernel_spmd(nc, in_maps, core_ids=list(range(NCORES)))

    out = np.empty((TOKENS, OUT_F), dtype=np.float32)
    for c in range(NCORES):
        out[NTOK * c:NTOK * (c + 1), :] = res.results[c]["out"].T
    return out


if __name__ == "__main__":
    rng = np.random.default_rng(0)
    x = rng.standard_normal((TOKENS, IN_F)).astype(np.float32)
    sb = (rng.standard_normal((8192, 16, 16)) * 0.02).astype(np.float32)
    bri = (np.arange(257) * 32).astype(np.int32)
    bci = np.sort(
        rng.integers(0, 256, size=(256, 32)), axis=1).reshape(-1).astype(np.int32)
    bias = (rng.standard_normal(4096) * 0.01).astype(np.float32)
    out = kernel(x, sb, bri, bci, bias)
    print("kernel ran, out shape", out.shape)


def prepare(inputs):
    """Build program + in_maps for external timing/tracing harnesses."""
    sched, w_host, bias_img = _preprocess(
        np.asarray(inputs["sparse_blocks"], dtype=np.float32),
        np.asarray(inputs["block_row_indices"], dtype=np.int32),
        np.asarray(inputs["block_col_indices"], dtype=np.int32),
        np.asarray(inputs["bias"], dtype=np.float32))
    nc = _build_program(sched, w_host.shape[2])
    x = np.asarray(inputs["x"], dtype=np.float32)
    in_maps = []
    for c in range(NCORES):
        xt = np.ascontiguousarray(x[NTOK * c:NTOK * (c + 1), :].T)
        in_maps.append({"x": xt, "w": w_host, "bias": bias_img})
    return nc, in_maps


# revision 8
# speedup vs baseline: 2.3976x; 1.2519x over previous
"""Dense-densified BlockSparseLinear kernel for TRN2 (8 cores).

The sparse mapping is LDWEIGHTS-overhead-bound on the PE (each 16x16 block
needs its own weight load; measured 34ns/load -> ~250us). Densifying the
87.5%-sparse weights into a full [4096, 4096] fp16 matrix lets the PE run
full 128x128 tiles at peak (~115us streaming, FWL-amortized weight loads).

Sharding: tokens/2 x out-features/4 (core c -> tc=c%2, oc=c//2).
Per core: x^T slice as fp16 K-panels [32][128, 1024] (host-cast), W^T dense
slice [4096, 1024] fp16 partition-major (2KB DMA descriptors), out^T
[1024, 1024] fp16 (host upcasts). PE: 2 token-blocks x 8 PSUM banks x 32
accumulating matmuls [K=128, M=128, N=512]; K-outer loop so compute
pipelines with K-ordered DMA arrivals.
"""

import numpy as np

BLOCK = 16
IN_F = 4096
OUT_F = 4096
TOKENS = 2048
NCORES = 8
TOK_SHARD = 2
OUT_SHARD = 4
CTOK = TOKENS // TOK_SHARD       # 1024 tokens per core
COUT = OUT_F // OUT_SHARD        # 1024 out features per core
NK = IN_F // 128                 # 32 K panels
NOT = COUT // 128                # 8 out tiles
NTB = CTOK // 512                # 2 token blocks of N=512


def _densify(sparse_blocks, block_row_indices, block_col_indices):
    """CSR 16x16 blocks -> dense W^T [IN_F, OUT_F] fp32 + presence matrix."""
    nnz = sparse_blocks.shape[0]
    row_ids = (
        np.searchsorted(
            block_row_indices, np.arange(nnz, dtype=block_row_indices.dtype),
            side="right",
        ) - 1
    ).astype(np.int64)
    cols = block_col_indices.astype(np.int64)
    wt = np.zeros((IN_F, OUT_F), dtype=np.float32)
    pres = np.zeros((OUT_F // 16, IN_F // 16), dtype=bool)
    # W^T[in, out] += block[out, in].T at (16*col, 16*row)
    for idx in range(nnz):
        r, c = row_ids[idx], cols[idx]
        wt[16 * c:16 * c + 16, 16 * r:16 * r + 16] += sparse_blocks[idx].T
        pres[r, c] = True
    return wt, pres


def _cluster_rows(pres):
    """Greedily group the 256 block-rows into 32 groups of 8 sharing column
    support, so [128,128] W tiles whose group avoids a K-panel's 8 block-cols
    can be skipped. Returns (groups [32][8], panel_mask [32 groups, 32 panels])."""
    nrows, ncols = pres.shape
    npan = ncols // 8
    panel_hit = pres.reshape(nrows, npan, 8).any(axis=2)  # row x k-panel
    remaining = set(range(nrows))
    groups = []
    while remaining:
        seed = max(remaining, key=lambda r: panel_hit[r].sum())
        grp = [seed]
        remaining.discard(seed)
        union = panel_hit[seed].copy()
        for _ in range(7):
            # candidate adding fewest NEW panels to the union
            best, best_new = None, None
            for r in remaining:
                new = int((~union & panel_hit[r]).sum())
                if best_new is None or new < best_new:
                    best, best_new = r, new
            grp.append(best)
            remaining.discard(best)
            union |= panel_hit[best]
        groups.append(grp)
    mask = np.zeros((len(groups), npan), dtype=bool)
    for g, grp in enumerate(groups):
        mask[g] = panel_hit[grp].any(axis=0)
    return groups, mask


def _build_program(core_mask):
    """core_mask: [NOT groups, NK panels] bool -- which (ot, k) tiles exist.
    Identical for all cores by construction (same 8 groups per out-shard
    pattern is NOT guaranteed, so we take the union across shards)."""
    import concourse.mybir as mybir
    from concourse import tile, bacc

    DT = mybir.dt.float32
    DTH = mybir.dt.float16

    nc = bacc.Bacc(None, target_bir_lowering=False, debug=False)
    # x^T slice, fp16, K-panel-major: [NK, 128, CTOK]
    x_dram = nc.dram_tensor("x", [NK, 128, CTOK], DTH, kind="ExternalInput")
    # W^T slice, fp16, partition-major: [128, NK*NOT*128] cols = (k, ot, q)
    w_dram = nc.dram_tensor("w", [128, NK * NOT * 128], DTH,
                            kind="ExternalInput")
    b_dram = nc.dram_tensor("bias", [128, NOT], DT, kind="ExternalInput")
    out_dram = nc.dram_tensor("out", [COUT, CTOK], DTH, kind="ExternalOutput")

    WSEG = NOT * 128  # W cols per k panel

    with tile.TileContext(nc) as tc:
        with (
            tc.tile_pool(name="wpool", bufs=1) as wpool,
            tc.tile_pool(name="fpool", bufs=NK) as fpool,
            tc.tile_pool(name="opool", bufs=8) as opool,
            tc.tile_pool(name="psum", bufs=1, space="PSUM") as psum_pool,
        ):
            w_sb = wpool.tile([128, NK * WSEG], DTH, tag="w")
            bias_sb = wpool.tile([128, NOT], DT, tag="bias")
            nc.sync.dma_start(bias_sb[:], b_dram[:])

            # K-ordered interleave of W k-segments and x panels; every piece
            # is 256KB with 2KB-per-partition descriptors
            xf = [None] * NK
            for k in range(NK):
                nsp = 4 if k < 1 else 1
                for sp in range(nsp):
                    c0, c1 = WSEG * sp // nsp, WSEG * (sp + 1) // nsp
                    nc.sync.dma_start(
                        w_sb[:, k * WSEG + c0:k * WSEG + c1],
                        w_dram[:, k * WSEG + c0:k * WSEG + c1])
                xk = fpool.tile([128, CTOK], DTH, tag="xf", name=f"xf{k}")
                for sp in range(nsp):
                    c0, c1 = CTOK * sp // nsp, CTOK * (sp + 1) // nsp
                    nc.sync.dma_start(xk[:, c0:c1], x_dram[k][:, c0:c1])
                xf[k] = xk

            psum_tiles = [psum_pool.tile([128, 512], DT, tag=f"ps{b}",
                                         name=f"ps{b}")
                          for b in range(8)]

            klists = [[k for k in range(NK) if core_mask[ot, k]]
                      for ot in range(NOT)]

            def drain(tb, ot):
                osb = opool.tile([128, 512], DTH, tag="o", name=f"o{tb}_{ot}")
                nc.scalar.activation(
                    osb[:], psum_tiles[ot][:],
                    mybir.ActivationFunctionType.Identity,
                    bias=bias_sb[:, ot:ot + 1],
                )
                for h in range(2):
                    nc.sync.dma_start(
                        out_dram[128 * ot:128 * (ot + 1),
                                 512 * tb + 256 * h:
                                 512 * tb + 256 * (h + 1)],
                        osb[:, 256 * h:256 * (h + 1)],
                    )

            # tb=0: K-outer (pipelines with K-ordered DMA arrivals)
            for k in range(NK):
                for ot in range(NOT):
                    if not core_mask[ot, k]:
                        continue
                    nc.tensor.matmul(
                        psum_tiles[ot][:, :],
                        w_sb[:, (k * NOT + ot) * 128:(k * NOT + ot + 1) * 128],
                        xf[k][:, 0:512],
                        start=(k == klists[ot][0]), stop=(k == klists[ot][-1]),
                    )
            for ot in range(NOT):
                drain(0, ot)
            # tb=1: OT-outer so drains pipeline with later banks' matmuls
            for ot in range(NOT):
                for k in klists[ot]:
                    nc.tensor.matmul(
                        psum_tiles[ot][:, :],
                        w_sb[:, (k * NOT + ot) * 128:(k * NOT + ot + 1) * 128],
                        xf[k][:, 512:1024],
                        start=(k == klists[ot][0]), stop=(k == klists[ot][-1]),
                    )
                drain(1, ot)

    nc.compile()
    return nc


def _axon_usable():
    import os
    jp = os.environ.get("JAX_PLATFORMS")
    if jp is not None and "axon" not in jp and jp != "":
        import sys
        if "jax" in sys.modules:
            try:
                import jax
                return any(d.platform not in ("cpu",) for d in jax.devices())
            except Exception:
                return False
        del os.environ["JAX_PLATFORMS"]
    try:
        import jax
        return any(d.platform not in ("cpu",) for d in jax.devices())
    except Exception:
        return False


def prepare(inputs):
    """Build program + per-core input maps. Returns (nc, in_maps, row_order)
    stored on the module for output unscrambling."""
    global _ROW_ORDER
    wt, pres = _densify(
        np.asarray(inputs["sparse_blocks"], dtype=np.float32),
        np.asarray(inputs["block_row_indices"], dtype=np.int32),
        np.asarray(inputs["block_col_indices"], dtype=np.int32))
    bias = np.asarray(inputs["bias"], dtype=np.float32)
    x = np.asarray(inputs["x"], dtype=np.float32)

    groups, mask = _cluster_rows(pres)      # 32 groups of 8 rows
    # SPMD: one program for all cores -> per (ot position) mask must match
    # across the 4 out-shards. Sort groups by panel count, deal them
    # round-robin to shards, and take the per-position union.
    order = np.argsort([-int(m.sum()) for m in mask], kind="stable")
    shard_groups = [[] for _ in range(OUT_SHARD)]
    for idx, g in enumerate(order):
        shard_groups[idx % OUT_SHARD].append(int(g))
    core_mask = np.zeros((NOT, NK), dtype=bool)
    for sg in shard_groups:
        for pos, g in enumerate(sg):
            core_mask[pos] |= mask[g]

    # permuted row order: shard oc, position pos -> 8 block-rows
    row_order = np.zeros((OUT_SHARD, NOT, 8), dtype=np.int64)
    for oc in range(OUT_SHARD):
        for pos, g in enumerate(shard_groups[oc]):
            row_order[oc, pos] = groups[g]
    _ROW_ORDER = row_order

    nc = _build_program(core_mask)
    in_maps = []
    for c in range(NCORES):
        tc_, oc = c % TOK_SHARD, c // TOK_SHARD
        xt = np.ascontiguousarray(
            x[CTOK * tc_:CTOK * (tc_ + 1), :].T.astype(np.float16)
        ).reshape(NK, 128, CTOK)
        # gather the shard's permuted 1024 out-features
        feats = (row_order[oc].reshape(-1, 1) * 16
                 + np.arange(16)).reshape(-1)          # [1024]
        wslice = wt[:, feats].astype(np.float16)
        wck = np.ascontiguousarray(
            wslice.reshape(NK, 128, NOT, 128).transpose(1, 0, 2, 3)
        ).reshape(128, NK * NOT * 128)
        bimg = np.ascontiguousarray(bias[feats].reshape(NOT, 128).T)
        in_maps.append({"x": xt, "w": wck, "bias": bimg})
    return nc, in_maps


_ROW_ORDER = None


def kernel(x, sparse_blocks, block_row_indices, block_col_indices, bias):
    x = np.asarray(x, dtype=np.float32)
    sparse_blocks = np.asarray(sparse_blocks, dtype=np.float32)
    block_row_indices = np.asarray(block_row_indices, dtype=np.int32)
    block_col_indices = np.asarray(block_col_indices, dtype=np.int32)
    bias = np.asarray(bias, dtype=np.float32)

    if not _axon_usable():
        import os
        import subprocess
        import sys
        import tempfile
        tmpdir = tempfile.mkdtemp(prefix="bslin_")
        in_npz = os.path.join(tmpdir, "in.npz")
        out_npz = os.path.join(tmpdir, "out.npz")
        np.savez(in_npz, x=x, sparse_blocks=sparse_blocks,
                 block_row_indices=block_row_indices,
                 block_col_indices=block_col_indices, bias=bias)
        code = (
            "import numpy as np, importlib.util;"
            "spec = importlib.util.spec_from_file_location('bslin_kernel', %r);"
            "m = importlib.util.module_from_spec(spec);"
            "spec.loader.exec_module(m);"
            "d = dict(np.load(%r));"
            "np.savez(%r, out=m.kernel(**d))" % (__file__, in_npz, out_npz)
        )
        env = {k: v for k, v in os.environ.items() if k != "JAX_PLATFORMS"}
        subprocess.run([sys.executable, "-c", code], check=True, env=env)
        return np.load(out_npz)["out"]

    nc, in_maps = prepare({
        "x": x, "sparse_blocks": sparse_blocks,
        "block_row_indices": block_row_indices,
        "block_col_indices": block_col_indices, "bias": bias})

    from concourse.bass_utils import run_bass_kernel_spmd
    res = run_bass_kernel_spmd(nc, in_maps, core_ids=list(range(NCORES)))

    out = np.empty((TOKENS, OUT_F), dtype=np.float32)
    for c in range(NCORES):
        tc_, oc = c % TOK_SHARD, c // TOK_SHARD
        feats = (_ROW_ORDER[oc].reshape(-1, 1) * 16
                 + np.arange(16)).reshape(-1)
        out[CTOK * tc_:CTOK * (tc_ + 1), feats] = (
            res.results[c]["out"].T.astype(np.float32))
    return out
